# revision 4
# baseline (speedup 1.0000x reference)
"""TRN2 Bass kernel for nn_LocalAggregation (gnn_message_passing).

Reference computation (per batch b, point n, neighbor k):
    pn = p[idx[n,k]]; dp = pn - p[n]                        # [3]
    arg[a,t] = 50*dp[a] / 500^(t/32)      (a<3, t<32)       # 96 args
    pe = [sin(arg) interleaved cos(arg)] per reference channel order
    agg = (x[:, idx[n,k]] + 1) * pe                          # [192]
    h = [dp; agg];  y = (W h) * inv + add;  out = max_k relu(y)

Mapping onto 8 NeuronCores: core c -> batch b=c//2, point half h=c%2 (2048 pts).

Dispatch is axon-tunneled (slow wire, ~40MB/s): minimize H2D/D2H bytes.
Three chained jits, intermediates stay device-resident:
  jit_prep: small raw inputs (x+1 bf16 channel-sharded over core pairs,
            p halves, idx i16, packed weights) -> all_gather within pairs
            -> gather table TH, wrapped indices IDXW, const block CB,
            zero output donor.  (XLA ops, compiles fast.)
  jit_exec: the Bass program (unchanged interface TH/IDXW/CB/CF -> OUT).
  jit_post: cast OUT f32 -> bf16 on device; fetch half the bytes.

Device pipeline per core (all matmuls bf16 with exactly-representable
selector weights; precision via bf16 hi/lo splits):
  - TH [4096, 256] bf16 per batch:
      elems 0..95   = (x+1)_hi for "sin block" channels (a*64+t)
      elems 128..223= (x+1)_hi for "cos block" channels (a*64+32+t)
      elems 96..122 = p split into 3 bf16 components (hi, mid, lo) x3 copies
  - dma_gather(transpose=True) -> slab [128, 2, FG]: channel-major tiles
  - mm_pn: ones-selector lhsT over the 9 p-component partitions -> psD [99, F]
  - mm_pc: -ones-selector over broadcast pc components (pT 3-way split) -> accum
  - ACT: pe_sin = Sin(psD * s), pe_cos = Sin(psD * s + pi/2)
  - DVE: agg = slab * pe (bf16, 2x mode); ACT: dp cast from psD
  - 4 bf16 matmuls (contraction 96 / 99, M-halves 128+64) -> psY [128, 1024]
  - DVE reduce max over k (32) -> [128, 2, 16]; ACT relu+bias -> out slab
"""

import os
import sys
import threading

import numpy as np

sys.path.insert(0, "/opt/trn_rl_repo")

import ml_dtypes

B, N, K, C = 4, 4096, 32, 192
FD = C // 6
EPS = 1e-5
NCORES = 8
NP = N // 2          # points per core
F = 512              # columns per sub-tile (16 points)
FG = 4096            # columns per gather slab (128 points)
NSUB = FG // F       # 8 sub-tiles per slab
NSLAB = NP * K // FG  # 16 slabs per core
PTS_SLAB = FG // K   # 128 points per slab
PTS_SUB = F // K     # 16 points per sub-tile

bf16 = ml_dtypes.bfloat16

_a96 = np.arange(96) // 32
_t96 = np.arange(96) % 32
C_SIN = (_a96 * 64 + _t96).astype(np.int32)   # orig x-channel for sin slot j
C_COS = (_a96 * 64 + 32 + _t96).astype(np.int32)
A99 = np.concatenate([_a96, np.arange(3)])  # axis index per psD partition

_dim_mat = np.power(np.float64(500.0), np.arange(FD, dtype=np.float64) / FD)
S96 = (50.0 / _dim_mat).astype(np.float32)[_t96]  # scale per arg slot
# turns-per-unit-d: q = (s/2pi)*d + 100; sin(arg) = sin(2pi*(q - round(q)))
SP96 = (S96.astype(np.float64) / (2 * np.pi)).astype(np.float32)
MAGIC = float(1.5 * 2.0**23)  # fp32 round-to-nearest via (q+M)-M

# CB column offsets
O0, O1, O2, O3, O4 = NP, NP + 99, NP + 198, NP + 390, NP + 582


def _split3(x):
    """fp32 -> three bf16 components summing to ~fp32 precision."""
    h = x.astype(bf16)
    r = x - h.astype(np.float32)
    m = r.astype(bf16)
    l = (r - m.astype(np.float32)).astype(bf16)
    return h, m, l


def build_weights(W, gamma, beta, rmean, rvar):
    inv = (gamma / np.sqrt(rvar + EPS)).astype(np.float32)
    Wp = (W * inv[:, None]).astype(np.float32)   # [192, 195]
    add = (beta - rmean * inv).astype(np.float32)
    ly0 = Wp[:, 3 + C_SIN].T.astype(bf16)        # [96, 192]
    ly1 = np.zeros((99, 192), np.float32)
    # cos block negated: device computes -cos via sin(2pi*(|w| - 1/4))
    ly1[0:96] = -Wp[:, 3 + C_COS].T
    ly1[96:99] = Wp[:, 0:3].T
    ly1 = ly1.astype(bf16)
    # selector lhsTs.
    # w_pn is consumed as rhs slab[64:123] (rows 0..31 = x junk, rows 32..58 =
    # three copies of the 9 p components, copy u scaled by s'_u = split_u(s/2pi)).
    # w_pc mirrors it over the PT broadcast rows plus a +100 const row.
    sp = [c.astype(np.float32) for c in _split3(SP96)]
    w_pn = np.zeros((59, 99), np.float32)
    w_pc = np.zeros((28, 99), np.float32)
    for u in range(3):
        for va in range(9):
            a = va % 3
            sel = (A99[0:96] == a).astype(np.float32)
            w_pn[32 + 9 * u + va, 0:96] = sp[u] * sel
            w_pc[9 * u + va, 0:96] = -sp[u] * sel
    # dp rows (96..98): plain pn - pc from the u=0 copy, all three v comps
    for va in range(9):
        a = va % 3
        w_pn[32 + va, 96 + a] += 1.0
        w_pc[va, 96 + a] += -1.0
    w_pc[27, 0:96] = 100.0  # q shift (exact in bf16)
    badd = np.zeros((128, 2), np.float32)
    badd[:, 0] = add[0:128]
    badd[0:64, 1] = add[128:192]
    return dict(
        ly0=ly0,
        ly1=ly1,
        w_pn=w_pn.astype(bf16),
        w_pc=w_pc.astype(bf16),
        badd=badd,
    )


def _build_program():
    import concourse.bacc as bacc
    import concourse.mybir as mybir
    import concourse.tile as tile

    f32 = mybir.dt.float32
    bf = mybir.dt.bfloat16
    i16 = mybir.dt.int16
    AF = mybir.ActivationFunctionType

    nslab_run = int(os.environ.get("K_NSLAB", NSLAB))

    nc = bacc.Bacc("TRN2", target_bir_lowering=False, debug=False)
    TH = nc.dram_tensor("TH", [N, 256], bf, kind="ExternalInput")
    IDXW = nc.dram_tensor("IDXW", [NSLAB, 128, FG // 16], i16, kind="ExternalInput")
    CB = nc.dram_tensor("CB", [128, NP + 99 + 99 + 192 + 192], bf, kind="ExternalInput")
    CF = nc.dram_tensor("CF", [128, 3], f32, kind="ExternalInput")
    OUT = nc.dram_tensor("OUT", [192, NP], f32, kind="ExternalOutput")

    with tile.TileContext(nc) as tc:
        with (
            tc.tile_pool(name="const", bufs=1) as cp,
            tc.tile_pool(name="slab", bufs=3) as sp,
            tc.tile_pool(name="work", bufs=4) as wp,
            tc.tile_pool(name="outp", bufs=3) as op,
            tc.tile_pool(name="psd", bufs=2, space="PSUM") as ppd,
            tc.tile_pool(name="psy", bufs=3, space="PSUM") as ppy,
        ):
            cb = cp.tile([128, NP + 99 + 99 + 192 + 192], bf)
            nc.sync.dma_start(out=cb[:], in_=CB[:])
            pt = cb[0:28, 0:NP]
            w_pn = cb[64:123, O0:O1]
            w_pc = cb[0:28, O1:O2]
            ly0 = cb[0:96, O2:O3]
            ly1 = cb[0:99, O3:O4]
            cf = cp.tile([128, 3], f32)
            nc.sync.dma_start(out=cf[:], in_=CF[:])
            badd = cf[:, 1:3]
            neghp = cp.tile([96, 1], f32)
            nc.gpsimd.memset(neghp[:], float(-np.pi / 2))
            mgc = cp.tile([96, 1], f32)
            nc.gpsimd.memset(mgc[:], MAGIC)

            for g in range(nslab_run):
                idxt = sp.tile([128, FG // 16], i16, tag="idx")
                nc.sync.dma_start(out=idxt[:], in_=IDXW[g])
                gch = int(os.environ.get("K_GCH", 512))
                ng = FG // gch
                slab = sp.tile([128, ng, 2, gch], bf, tag="slab")
                for j in range(ng):
                    nc.gpsimd.dma_gather(
                        slab[:, j, :, :],
                        TH[:],
                        idxt[:, j * (gch // 16) : (j + 1) * (gch // 16)],
                        gch,
                        gch,
                        256,
                        transpose=True,
                    )
                outs = op.tile([128, 2, PTS_SLAB], f32, tag="outs")
                redslab = op.tile([128, 2, PTS_SLAB], f32, tag="redslab")
                for s in range(NSUB):
                    jj = (s * F) // gch
                    off = (s * F) % gch
                    cols = slice(off, off + F)
                    pt0 = g * PTS_SLAB + s * PTS_SUB
                    # d (replicated to 99 partitions) = pn - pc, fp32-exact
                    psd = ppd.tile([99, F], f32, tag="psd")
                    nc.tensor.matmul(
                        psd[:],
                        lhsT=w_pn,
                        rhs=slab[64:123, jj, 0, cols],
                        start=True,
                        stop=False,
                    )
                    pc_rhs = (
                        pt[:, pt0 : pt0 + PTS_SUB]
                        .rearrange("p (n o) -> p n o", o=1)
                        .to_broadcast([28, PTS_SUB, K])
                    )
                    nc.tensor.matmul(
                        psd[:], lhsT=w_pc, rhs=pc_rhs, start=False, stop=True
                    )
                    # psd rows 0..95 hold q = arg/(2pi) + 100.
                    # ACT's fp32 add rounds: t = fl(q + M) = M + round(q);
                    # GPSIMD: rr = t - M = round(q); DVE: w = q - rr.
                    tq = wp.tile([96, F], f32, tag="tq")
                    nc.scalar.activation(
                        tq[:], psd[0:96, :], AF.Identity, bias=mgc[:]
                    )
                    rr = wp.tile([96, F], f32, tag="rr")
                    nc.gpsimd.tensor_scalar(
                        rr[:], tq[:], -MAGIC, None, op0=mybir.AluOpType.add
                    )
                    ww = wp.tile([96, F], f32, tag="ww")
                    nc.vector.tensor_tensor(
                        out=ww[:], in0=psd[0:96, :], in1=rr[:],
                        op=mybir.AluOpType.subtract,
                    )
                    # wc = |2pi*w| (ACT Abs); sin(wc - pi/2) = -cos(arg)
                    wc = wp.tile([96, F], f32, tag="wc")
                    nc.scalar.activation(wc[:], ww[:], AF.Abs, scale=float(2 * np.pi))
                    # pe0 = sin(2pi*w) = sin(arg); pe1 = -cos(arg) (ly1 negated)
                    pe = wp.tile([96, 2, F], bf, tag="pe")
                    nc.scalar.activation(
                        pe[:, 0, :], ww[:], AF.Sin, scale=float(2 * np.pi)
                    )
                    nc.scalar.activation(pe[:, 1, :], wc[:], AF.Sin, bias=neghp[:])
                    # agg = slab_x * pe ; dp cast into agg[96:99, 1, :]
                    agg = wp.tile([99, 2, F], bf, tag="agg")
                    nc.vector.tensor_tensor(
                        out=agg[0:96, :, :],
                        in0=slab[0:96, jj, :, cols],
                        in1=pe[:],
                        op=mybir.AluOpType.mult,
                    )
                    nc.scalar.copy(agg[96:99, 1, :], psd[96:99, :])
                    # y matmuls: psY [128, 1024] = two 512-col M-half blocks
                    psy = ppy.tile([128, 1024], f32, tag="psy")
                    nc.tensor.matmul(
                        psy[:, 0:512],
                        lhsT=ly0[:, 0:128],
                        rhs=agg[0:96, 0, :],
                        start=True,
                        stop=False,
                    )
                    nc.tensor.matmul(
                        psy[:, 0:512],
                        lhsT=ly1[:, 0:128],
                        rhs=agg[:, 1, :],
                        start=False,
                        stop=True,
                    )
                    nc.tensor.matmul(
                        psy[0:64, 512:1024],
                        lhsT=ly0[:, 128:192],
                        rhs=agg[0:96, 0, :],
                        start=True,
                        stop=False,
                    )
                    nc.tensor.matmul(
                        psy[0:64, 512:1024],
                        lhsT=ly1[:, 128:192],
                        rhs=agg[:, 1, :],
                        start=False,
                        stop=True,
                    )
                    # reduce max over k
                    oc = slice(s * PTS_SUB, (s + 1) * PTS_SUB)
                    nc.vector.tensor_reduce(
                        redslab[:, 0, oc],
                        psy[:, 0:512].rearrange("p (n k) -> p n k", k=K),
                        axis=mybir.AxisListType.X,
                        op=mybir.AluOpType.max,
                    )
                    nc.vector.tensor_reduce(
                        redslab[0:64, 1, oc],
                        psy[0:64, 512:1024].rearrange("p (n k) -> p n k", k=K),
                        axis=mybir.AxisListType.X,
                        op=mybir.AluOpType.max,
                    )
                # relu + bias once per slab
                nc.scalar.activation(
                    outs[:, 0, :], redslab[:, 0, :], AF.Relu, bias=badd[:, 0:1]
                )
                nc.scalar.activation(
                    outs[0:64, 1, :], redslab[0:64, 1, :], AF.Relu,
                    bias=badd[0:64, 1:2],
                )
                nc.sync.dma_start(
                    out=OUT[0:128, g * PTS_SLAB : (g + 1) * PTS_SLAB],
                    in_=outs[:, 0, :],
                )
                nc.sync.dma_start(
                    out=OUT[128:192, g * PTS_SLAB : (g + 1) * PTS_SLAB],
                    in_=outs[0:64, 1, :],
                )
    nc.finalize()
    return nc


class _State:
    pass


_STATE = None
_STATE_LOCK = threading.Lock()


def _build_state():
    import jax
    import jax.numpy as jnp
    from jax.sharding import Mesh, PartitionSpec
    from jax.experimental.shard_map import shard_map

    import concourse.mybir as mybir
    from concourse import bass2jax
    from concourse.bass2jax import (
        _bass_exec_p,
        install_neuronx_cc_hook,
        partition_id_tensor,
    )

    install_neuronx_cc_hook()

    st = _State()
    nc = _build_program()
    st.nc = nc

    devices = jax.devices()[:NCORES]
    mesh = Mesh(np.asarray(devices), ("core",))
    st.mesh = mesh

    # ---- jit_exec: replicate run_bass_via_pjrt's _body, cached across calls
    partition_name = (
        nc.partition_id_tensor.name if nc.partition_id_tensor is not None else None
    )
    in_names = []
    out_names = []
    out_avals = []
    for alloc in nc.m.functions[0].allocations:
        if not isinstance(alloc, mybir.MemoryLocationSet):
            continue
        name = alloc.memorylocations[0].name
        if alloc.kind == "ExternalInput":
            if name != partition_name:
                in_names.append(name)
        elif alloc.kind == "ExternalOutput":
            out_names.append(name)
            shape = tuple(alloc.tensor_shape)
            dtype = mybir.dt.np(alloc.dtype)
            out_avals.append(jax.core.ShapedArray(shape, dtype))
    n_params = len(in_names)
    n_outs = len(out_avals)
    in_names = in_names + out_names
    if partition_name is not None:
        in_names.append(partition_name)
    st.in_names = tuple(in_names)
    st.out_avals = out_avals
    donate = tuple(range(n_params, n_params + n_outs))

    def _body(*args):
        operands = list(args)
        if partition_name is not None:
            operands.append(partition_id_tensor())
        outs = _bass_exec_p.bind(
            *operands,
            out_avals=tuple(out_avals),
            in_names=tuple(in_names),
            out_names=tuple(out_names),
            lowering_input_output_aliases=(),
            sim_require_finite=True,
            sim_require_nnan=True,
            nc=nc,
        )
        return tuple(outs)

    st.jit_exec = jax.jit(
        shard_map(
            _body,
            mesh=mesh,
            in_specs=(PartitionSpec("core"),) * (n_params + n_outs),
            out_specs=(PartitionSpec("core"),) * len(out_names),
            check_rep=False,
        ),
        donate_argnums=donate,
        keep_unused=True,
    )

    # ---- jit_prep: build TH / IDXW / CB / zero-donor on device
    csin = jnp.asarray(C_SIN)
    ccos = jnp.asarray(C_COS)

    def prep(xh, ph, idxh, wpk):
        # xh  [96, N] bf16: this core's channel half of its batch's (x+1)
        # ph  [NP, 3] f32: this core's half of its batch's points
        # idxh[NP, K] i16; wpk [99, 471] bf16 packed weights
        groups = [[0, 1], [2, 3], [4, 5], [6, 7]]
        xf = jax.lax.all_gather(
            xh, "core", axis_index_groups=groups, tiled=True
        )  # [192, N]
        pf = jax.lax.all_gather(
            ph, "core", axis_index_groups=groups, tiled=True
        )  # [N, 3]
        th = jnp.zeros((N, 256), jnp.bfloat16)
        th = th.at[:, 0:96].set(xf[csin, :].T)
        th = th.at[:, 128:224].set(xf[ccos, :].T)
        h = pf.astype(jnp.bfloat16)
        r = pf - h.astype(jnp.float32)
        m = r.astype(jnp.bfloat16)
        l = (r - m.astype(jnp.float32)).astype(jnp.bfloat16)
        comps9 = jnp.concatenate([h, m, l], axis=1)  # [N, 9]
        for u in range(3):
            th = th.at[:, 96 + 9 * u : 105 + 9 * u].set(comps9)
        # idx wrap: [NP*K] -> [NSLAB, 16, FG//16] -> tile 8x partitions
        w = idxh.reshape(NSLAB, FG // 16, 16).transpose(0, 2, 1)
        idxw = jnp.tile(w, (1, 8, 1))  # [NSLAB, 128, FG//16]
        # CB image
        cb = jnp.zeros((128, O4), jnp.bfloat16)
        # PT: split3 of own half points, transposed -> [9, NP], 3 copies + ones
        hh = ph.astype(jnp.bfloat16)
        rr = ph - hh.astype(jnp.float32)
        mm = rr.astype(jnp.bfloat16)
        ll = (rr - mm.astype(jnp.float32)).astype(jnp.bfloat16)
        c9 = jnp.concatenate([hh, mm, ll], axis=1).T  # [9, NP]
        pt = jnp.concatenate(
            [c9, c9, c9, jnp.ones((1, NP), jnp.bfloat16)], axis=0
        )  # [28, NP]
        cb = cb.at[0:28, 0:NP].set(pt)
        cb = cb.at[64:123, O0:O1].set(wpk[:, 0:59].T)
        cb = cb.at[0:28, O1:O2].set(wpk[:, 59:87].T)
        cb = cb.at[0:96, O2:O3].set(wpk[0:96, 87:279])
        cb = cb.at[0:99, O3:O4].set(wpk[:, 279:471])
        z = jnp.zeros((192, NP), jnp.float32)
        return th, idxw, cb, z

    st.jit_prep = jax.jit(
        shard_map(
            prep,
            mesh=mesh,
            in_specs=(PartitionSpec("core"),) * 4,
            out_specs=(PartitionSpec("core"),) * 4,
        )
    )

    # ---- jit_post: cast output f32 -> bf16 on device before fetch
    st.jit_post = jax.jit(
        shard_map(
            lambda o: o.astype(jnp.bfloat16),
            mesh=mesh,
            in_specs=(PartitionSpec("core"),),
            out_specs=PartitionSpec("core"),
        )
    )
    return st


def _get_state():
    global _STATE
    with _STATE_LOCK:
        if _STATE is None:
            _STATE = _build_state()
    return _STATE


def host_prep(p, x, idx, W, gamma, beta, rmean, rvar):
    """Cheap host-side packing -> small wire tensors."""
    p = np.ascontiguousarray(np.asarray(p, np.float32))
    x = np.asarray(x, np.float32)
    idx = np.asarray(idx)
    wd = build_weights(
        np.asarray(W, np.float32),
        np.asarray(gamma, np.float32),
        np.asarray(beta, np.float32),
        np.asarray(rmean, np.float32),
        np.asarray(rvar, np.float32),
    )
    wpk = np.zeros((99, 471), bf16)
    wpk[:, 0:59] = wd["w_pn"].T
    wpk[:, 59:87] = wd["w_pc"].T
    wpk[0:96, 87:279] = wd["ly0"]
    wpk[:, 279:471] = wd["ly1"]
    cf = np.zeros((128, 3), np.float32)
    cf[:, 1:3] = wd["badd"]

    xp = (x + np.float32(1.0)).astype(bf16)  # [B, C, N]
    # core c: batch c//2, channel half c%2 for the all-gather; point half c%2
    xh8 = np.empty((NCORES * 96, N), bf16)
    ph8 = np.empty((NCORES * NP, 3), np.float32)
    id8 = np.empty((NCORES * NP, K), np.int16)
    for c in range(NCORES):
        b, h = c // 2, c % 2
        xh8[c * 96 : (c + 1) * 96] = xp[b, h * 96 : (h + 1) * 96, :]
        ph8[c * NP : (c + 1) * NP] = p[b, h * NP : (h + 1) * NP]
        id8[c * NP : (c + 1) * NP] = idx[b, h * NP : (h + 1) * NP].astype(np.int16)
    wpk8 = np.broadcast_to(wpk, (NCORES, 99, 471)).reshape(NCORES * 99, 471)
    cf8 = np.broadcast_to(cf, (NCORES, 128, 3)).reshape(NCORES * 128, 3)
    return xh8, ph8, id8, np.ascontiguousarray(wpk8), np.ascontiguousarray(cf8)


def device_run(st, xh8, ph8, id8, wpk8, cf8):
    """The timed dispatch: prep -> bass exec -> cast -> fetch."""
    th, idxw, cbb, z = st.jit_prep(xh8, ph8, id8, wpk8)
    (out,) = st.jit_exec(th, idxw, cbb, cf8, z)
    ob = st.jit_post(out)
    return np.asarray(ob)


def assemble(ob):
    out = np.zeros((B, C, N), np.float32)
    o = ob.astype(np.float32).reshape(NCORES, C, NP)
    for c in range(NCORES):
        b, h = c // 2, c % 2
        out[b, :, h * NP : (h + 1) * NP] = o[c]
    return out


def kernel(p, x, idx, W, gamma, beta, rmean, rvar):
    st = _get_state()
    args = host_prep(p, x, idx, W, gamma, beta, rmean, rvar)
    ob = device_run(st, *args)
    return assemble(ob)


if __name__ == "__main__":
    pass
